# revision 15
# baseline (speedup 1.0000x reference)
"""Trainium2 (Bass/Tile) kernel for the DTI PU loss.

loss = (1-a)/2 * sum_pos (R-P)[x,y]^2  +  a/2 * sum_neg (R-P)[x,y]^2

The reference is "equivalent to dense MSE matrix followed by fancy
indexing" (its own words).  The memory-roofline formulation of that is a
dense weighted MSE:

    loss = sum_cells  W[i,j] * (R[i,j] - P[i,j])^2
    W    = (1-a)/2 * count_pos + a/2 * count_neg

Sharding (8 NeuronCores, data-parallel by row blocks, per the hint):
  * Host shards R, P by 1024-row blocks and folds each core's shard of
    the index lists into a dense fp16 weight image W (a bincount) —
    index preprocessing on the host, weighted reduction on the device.
  * Per core the device streams R (32 MB), P (32 MB) and W (16 MB) from
    HBM in [128, 4096] tiles and computes
        acc += sum( fp16(R - P)^2 * W )
    with DVE subtract, ACT square, and a fused multiply+reduce
    (scalar_tensor_tensor) into per-partition fp32 accumulators.
    That is ~80 MB of HBM traffic per core => ~240 us at ~330 GB/s.
  * Host sums the 8 [128] partial-sum vectors (the "all-reduce").
"""

import numpy as np

# ---------------------------------------------------------------- constants
N_FULL = 8192
M_FULL = 8192
N_CORES = 8
ROWS_PER_CORE = N_FULL // N_CORES            # 1024
N_BLK = ROWS_PER_CORE // 128                 # 8 partition blocks per core
COL_CHUNK = 4096
N_CC = M_FULL // COL_CHUNK                   # column chunks per block


# ---------------------------------------------------------------- host prep
def _weight_image(inputs):
    """Fold the index lists + alpha into a dense fp16 weight matrix."""
    a = float(np.asarray(inputs["alpha"]).reshape(-1)[0])
    wp = (1.0 - a) * 0.5
    wn = a * 0.5
    ncell = N_FULL * M_FULL

    def counts(xk, yk):
        x = np.asarray(inputs[xk], dtype=np.int64)
        y = np.asarray(inputs[yk], dtype=np.int64)
        return np.bincount(x * M_FULL + y, minlength=ncell)

    cpos = counts("pos_x_index", "pos_y_index")
    cneg = counts("neg_x_index", "neg_y_index")
    w = np.sqrt(
        wp * cpos.astype(np.float32) + wn * cneg.astype(np.float32)
    ).astype(np.float16)
    return w.reshape(N_FULL, M_FULL)


def _prepare(inputs):
    R = np.ascontiguousarray(
        np.asarray(inputs["drug_protein_reconstruct"], dtype=np.float32).astype(
            np.float16
        )
    )
    P = np.ascontiguousarray(
        np.asarray(inputs["drug_protein"], dtype=np.float32).astype(np.float16)
    )
    W = _weight_image(inputs)
    in_maps = []
    for c in range(N_CORES):
        rows = slice(c * ROWS_PER_CORE, (c + 1) * ROWS_PER_CORE)
        in_maps.append({"r": R[rows], "p": P[rows], "w": W[rows]})
    return in_maps


# ---------------------------------------------------------------- device IR
def _build_program(enable_asserts=False):
    from contextlib import ExitStack

    import concourse.bacc as bacc
    import concourse.mybir as mybir
    import concourse.tile as tile

    f32 = mybir.dt.float32
    f16 = mybir.dt.float16

    nc = bacc.Bacc(
        "TRN2",
        target_bir_lowering=False,
        debug=False,
        enable_asserts=enable_asserts,
        num_devices=N_CORES,
    )
    r_d = nc.dram_tensor("r", [ROWS_PER_CORE, M_FULL], f16, kind="ExternalInput").ap()
    p_d = nc.dram_tensor("p", [ROWS_PER_CORE, M_FULL], f16, kind="ExternalInput").ap()
    w_d = nc.dram_tensor("w", [ROWS_PER_CORE, M_FULL], f16, kind="ExternalInput").ap()
    acc_d = nc.dram_tensor("acc", [128, 1], f32, kind="ExternalOutput").ap()

    n_tiles = N_BLK * N_CC

    with tile.TileContext(nc) as tc, ExitStack() as ctx:
        rp = ctx.enter_context(tc.tile_pool(name="rp", bufs=5))
        wp_ = ctx.enter_context(tc.tile_pool(name="wp", bufs=5))
        dp = ctx.enter_context(tc.tile_pool(name="dp", bufs=4))
        sp = ctx.enter_context(tc.tile_pool(name="sp", bufs=2))
        accs = ctx.enter_context(tc.tile_pool(name="accs", bufs=1))

        accc = accs.tile([128, n_tiles], f32)
        ti = 0
        for blk in range(N_BLK):
            rows = slice(blk * 128, (blk + 1) * 128)
            for cc in range(N_CC):
                cols = slice(cc * COL_CHUNK, (cc + 1) * COL_CHUNK)
                rt = rp.tile([128, COL_CHUNK], f16, tag="rt")
                nc.sync.dma_start(out=rt[:], in_=r_d[rows, cols])
                pt = rp.tile([128, COL_CHUNK], f16, tag="pt")
                nc.sync.dma_start(out=pt[:], in_=p_d[rows, cols])
                wt = wp_.tile([128, COL_CHUNK], f16, tag="wt")
                nc.scalar.dma_start(out=wt[:], in_=w_d[rows, cols])

                dt = dp.tile([128, COL_CHUNK], f16, tag="dt")
                sub_eng = nc.gpsimd if (ti % 4 == 0) else nc.vector
                sub_eng.tensor_sub(dt[:], rt[:], pt[:])
                mul_eng = nc.gpsimd if (ti % 4 == 2) else nc.vector
                mul_eng.tensor_mul(dt[:], dt[:], wt[:])
                st = sp.tile([128, COL_CHUNK], f16, tag="st")
                nc.scalar.activation(
                    st[:],
                    dt[:],
                    mybir.ActivationFunctionType.Square,
                    accum_out=accc[:, ti : ti + 1],
                )
                ti += 1

        accf = accs.tile([128, 1], f32)
        nc.vector.tensor_reduce(
            accf[:], accc[:], axis=mybir.AxisListType.X, op=mybir.AluOpType.add
        )
        nc.sync.dma_start(out=acc_d[:], in_=accf[:])

    nc.compile()
    return nc


def _combine(result_maps):
    tot = 0.0
    for m in result_maps:
        tot += float(np.asarray(m["acc"], dtype=np.float64).sum())
    return np.asarray(tot, dtype=np.float32)


_LAST_RESULTS = {}


def kernel(**inputs):
    from concourse.bass_utils import run_bass_kernel_spmd

    in_maps = _prepare(inputs)
    nc = _build_program()
    res = run_bass_kernel_spmd(nc, in_maps, list(range(N_CORES)))
    _LAST_RESULTS["res"] = res
    return _combine(res.results)


# ---------------------------------------------------------------- sim check
def _sim_check(n_pos=60000, n_neg=200000, seed=0):
    from concourse.bass_interp import CoreSim

    rng = np.random.default_rng(seed)
    R = rng.standard_normal((N_FULL, M_FULL), dtype=np.float32)
    P = rng.random((N_FULL, M_FULL), dtype=np.float32)
    inputs = {
        "drug_protein_reconstruct": R,
        "drug_protein": P,
        "alpha": np.array([0.3], np.float32),
        "pos_x_index": rng.integers(0, N_FULL, n_pos),
        "pos_y_index": rng.integers(0, M_FULL, n_pos),
        "neg_x_index": rng.integers(0, N_FULL, n_neg),
        "neg_y_index": rng.integers(0, M_FULL, n_neg),
    }
    in_maps = _prepare(inputs)
    nc = _build_program(enable_asserts=True)
    sim = CoreSim(nc)
    for name, arr in in_maps[0].items():
        sim.tensor(name)[:] = arr
    sim.simulate()
    acc = float(np.asarray(sim.tensor("acc"), np.float64).sum())

    a = 0.3
    wp, wn = (1 - a) / 2, a / 2
    Rb = R[:ROWS_PER_CORE].astype(np.float64)
    Pb = P[:ROWS_PER_CORE].astype(np.float64)
    S = (Rb - Pb) ** 2
    exp = 0.0
    for w, xk, yk in ((wp, "pos_x_index", "pos_y_index"),
                      (wn, "neg_x_index", "neg_y_index")):
        xs = np.asarray(inputs[xk])
        ys = np.asarray(inputs[yk])
        sel = xs < ROWS_PER_CORE
        exp += w * S[xs[sel], ys[sel]].sum()
    rel = abs(acc - exp) / exp
    print(f"core0: got={acc:.6f} exp={exp:.6f} relerr={rel:.2e}")
    assert rel < 5e-3
    print("SIM CHECK PASSED")


if __name__ == "__main__":
    import sys

    if "--sim" in sys.argv:
        _sim_check()


# revision 16
# speedup vs baseline: 1.0090x; 1.0090x over previous
"""Trainium2 (Bass/Tile) kernel for the DTI PU loss.

loss = (1-a)/2 * sum_pos (R-P)[x,y]^2  +  a/2 * sum_neg (R-P)[x,y]^2

The reference is "equivalent to dense MSE matrix followed by fancy
indexing" (its own words).  The memory-roofline formulation of that is a
dense weighted MSE:

    loss = sum_cells  W[i,j] * (R[i,j] - P[i,j])^2
    W    = (1-a)/2 * count_pos + a/2 * count_neg

Sharding (8 NeuronCores, data-parallel by row blocks, per the hint):
  * Host shards R, P by 1024-row blocks and folds each core's shard of
    the index lists into a dense fp16 weight image W (a bincount) —
    index preprocessing on the host, weighted reduction on the device.
  * Per core the device streams R (32 MB), P (32 MB) and W (16 MB) from
    HBM in [128, 4096] tiles and computes
        acc += sum( fp16(R - P)^2 * W )
    with DVE subtract, ACT square, and a fused multiply+reduce
    (scalar_tensor_tensor) into per-partition fp32 accumulators.
    That is ~80 MB of HBM traffic per core => ~240 us at ~330 GB/s.
  * Host sums the 8 [128] partial-sum vectors (the "all-reduce").
"""

import numpy as np

# ---------------------------------------------------------------- constants
N_FULL = 8192
M_FULL = 8192
N_CORES = 8
ROWS_PER_CORE = N_FULL // N_CORES            # 1024
N_BLK = ROWS_PER_CORE // 128                 # 8 partition blocks per core
COL_CHUNK = 4096
N_CC = M_FULL // COL_CHUNK                   # column chunks per block


# ---------------------------------------------------------------- host prep
def _weight_image(inputs):
    """Fold the index lists + alpha into a dense fp16 weight matrix."""
    a = float(np.asarray(inputs["alpha"]).reshape(-1)[0])
    wp = (1.0 - a) * 0.5
    wn = a * 0.5
    ncell = N_FULL * M_FULL

    def counts(xk, yk):
        x = np.asarray(inputs[xk], dtype=np.int64)
        y = np.asarray(inputs[yk], dtype=np.int64)
        return np.bincount(x * M_FULL + y, minlength=ncell)

    cpos = counts("pos_x_index", "pos_y_index")
    cneg = counts("neg_x_index", "neg_y_index")
    w = np.sqrt(
        wp * cpos.astype(np.float32) + wn * cneg.astype(np.float32)
    ).astype(np.float16)
    return w.reshape(N_FULL, M_FULL)


def _prepare(inputs):
    R = np.ascontiguousarray(
        np.asarray(inputs["drug_protein_reconstruct"], dtype=np.float32).astype(
            np.float16
        )
    )
    P = np.ascontiguousarray(
        np.asarray(inputs["drug_protein"], dtype=np.float32).astype(np.float16)
    )
    W = _weight_image(inputs)
    in_maps = []
    for c in range(N_CORES):
        rows = slice(c * ROWS_PER_CORE, (c + 1) * ROWS_PER_CORE)
        in_maps.append({"r": R[rows], "p": P[rows], "w": W[rows]})
    return in_maps


# ---------------------------------------------------------------- device IR
def _build_program(enable_asserts=False):
    from contextlib import ExitStack

    import concourse.bacc as bacc
    import concourse.mybir as mybir
    import concourse.tile as tile

    f32 = mybir.dt.float32
    f16 = mybir.dt.float16

    nc = bacc.Bacc(
        "TRN2",
        target_bir_lowering=False,
        debug=False,
        enable_asserts=enable_asserts,
        num_devices=N_CORES,
    )
    r_d = nc.dram_tensor("r", [ROWS_PER_CORE, M_FULL], f16, kind="ExternalInput").ap()
    p_d = nc.dram_tensor("p", [ROWS_PER_CORE, M_FULL], f16, kind="ExternalInput").ap()
    w_d = nc.dram_tensor("w", [ROWS_PER_CORE, M_FULL], f16, kind="ExternalInput").ap()
    acc_d = nc.dram_tensor("acc", [128, 1], f32, kind="ExternalOutput").ap()

    n_tiles = N_BLK * N_CC

    with tile.TileContext(nc) as tc, ExitStack() as ctx:
        rp = ctx.enter_context(tc.tile_pool(name="rp", bufs=4))
        wp_ = ctx.enter_context(tc.tile_pool(name="wp", bufs=4))
        dp = ctx.enter_context(tc.tile_pool(name="dp", bufs=3))
        sp = ctx.enter_context(tc.tile_pool(name="sp", bufs=2))
        accs = ctx.enter_context(tc.tile_pool(name="accs", bufs=1))

        accc = accs.tile([128, n_tiles], f32)
        ti = 0
        for blk in range(N_BLK):
            rows = slice(blk * 128, (blk + 1) * 128)
            for cc in range(N_CC):
                cols = slice(cc * COL_CHUNK, (cc + 1) * COL_CHUNK)
                rt = rp.tile([128, COL_CHUNK], f16, tag="rt")
                nc.sync.dma_start(out=rt[:], in_=r_d[rows, cols])
                pt = rp.tile([128, COL_CHUNK], f16, tag="pt")
                nc.sync.dma_start(out=pt[:], in_=p_d[rows, cols])
                wt = wp_.tile([128, COL_CHUNK], f16, tag="wt")
                nc.scalar.dma_start(out=wt[:], in_=w_d[rows, cols])

                dt = dp.tile([128, COL_CHUNK], f16, tag="dt")
                sub_eng = nc.gpsimd if (ti % 4 == 0) else nc.vector
                sub_eng.tensor_sub(dt[:], rt[:], pt[:])
                mul_eng = nc.gpsimd if (ti % 4 == 2) else nc.vector
                mul_eng.tensor_mul(dt[:], dt[:], wt[:])
                st = sp.tile([128, COL_CHUNK], f16, tag="st")
                nc.scalar.activation(
                    st[:],
                    dt[:],
                    mybir.ActivationFunctionType.Square,
                    accum_out=accc[:, ti : ti + 1],
                )
                ti += 1

        accf = accs.tile([128, 1], f32)
        nc.vector.tensor_reduce(
            accf[:], accc[:], axis=mybir.AxisListType.X, op=mybir.AluOpType.add
        )
        nc.sync.dma_start(out=acc_d[:], in_=accf[:])

    nc.compile()
    return nc


def _combine(result_maps):
    tot = 0.0
    for m in result_maps:
        tot += float(np.asarray(m["acc"], dtype=np.float64).sum())
    return np.asarray(tot, dtype=np.float32)


_LAST_RESULTS = {}


def kernel(**inputs):
    from concourse.bass_utils import run_bass_kernel_spmd

    in_maps = _prepare(inputs)
    nc = _build_program()
    res = run_bass_kernel_spmd(nc, in_maps, list(range(N_CORES)))
    _LAST_RESULTS["res"] = res
    return _combine(res.results)


# ---------------------------------------------------------------- sim check
def _sim_check(n_pos=60000, n_neg=200000, seed=0):
    from concourse.bass_interp import CoreSim

    rng = np.random.default_rng(seed)
    R = rng.standard_normal((N_FULL, M_FULL), dtype=np.float32)
    P = rng.random((N_FULL, M_FULL), dtype=np.float32)
    inputs = {
        "drug_protein_reconstruct": R,
        "drug_protein": P,
        "alpha": np.array([0.3], np.float32),
        "pos_x_index": rng.integers(0, N_FULL, n_pos),
        "pos_y_index": rng.integers(0, M_FULL, n_pos),
        "neg_x_index": rng.integers(0, N_FULL, n_neg),
        "neg_y_index": rng.integers(0, M_FULL, n_neg),
    }
    in_maps = _prepare(inputs)
    nc = _build_program(enable_asserts=True)
    sim = CoreSim(nc)
    for name, arr in in_maps[0].items():
        sim.tensor(name)[:] = arr
    sim.simulate()
    acc = float(np.asarray(sim.tensor("acc"), np.float64).sum())

    a = 0.3
    wp, wn = (1 - a) / 2, a / 2
    Rb = R[:ROWS_PER_CORE].astype(np.float64)
    Pb = P[:ROWS_PER_CORE].astype(np.float64)
    S = (Rb - Pb) ** 2
    exp = 0.0
    for w, xk, yk in ((wp, "pos_x_index", "pos_y_index"),
                      (wn, "neg_x_index", "neg_y_index")):
        xs = np.asarray(inputs[xk])
        ys = np.asarray(inputs[yk])
        sel = xs < ROWS_PER_CORE
        exp += w * S[xs[sel], ys[sel]].sum()
    rel = abs(acc - exp) / exp
    print(f"core0: got={acc:.6f} exp={exp:.6f} relerr={rel:.2e}")
    assert rel < 5e-3
    print("SIM CHECK PASSED")


if __name__ == "__main__":
    import sys

    if "--sim" in sys.argv:
        _sim_check()
